# revision 1
# baseline (speedup 1.0000x reference)
"""Equivariant MLP (9 -> 49 -> 49 -> 9, tied weights) on 8 trn2 NeuronCores.

Data parallel over the batch (1048576 rows -> 131072/core).  Tied-weight
patterns are expanded to dense matrices on the host (tiny gathers).  The
device runs feature-major: the host hands each core x^T as a banded
[4, 18, 16384] array — band j holds batch-block pair (2j, 2j+1) stacked on
9+9 partitions — which the kernel DMAs into SBUF partitions {32j..32j+17}
(32-aligned so each pair can be a matmul operand).  Per pair: L1 matmul with
block-diagonal [18, 98] weights, fused bias+relu on ACT (PSUM->SBUF), L2
[98, 98] matmul, fused bias+relu on DVE, then L3 as four accumulating
matmuls with zero-padded [98, 72] weights packing all four pairs' y^T into
one [72, C] PSUM tile (one cheap evacuation).  Matmuls use float32r (fp32
bits, single-pass PE mode: 1 cycle/column vs 4 for strict fp32).
"""

import os
import sys

sys.path.insert(0, "/opt/trn_rl_repo")

import numpy as np

import concourse.bass as bass
import concourse.mybir as mybir
import concourse.tile as tile
from concourse.bass_utils import run_bass_kernel_spmd

f32 = mybir.dt.float32
f32r = mybir.dt.float32r

N_CORES = 8
BATCH = 1048576
BS = BATCH // N_CORES          # 131072 rows per core
NBLK = 8                       # batch blocks per core (4 pairs)
S = BS // NBLK                 # 16384 columns per block
C = 1024                       # columns per strip (DMA + evac width)
MM = 512                       # columns per matmul (PSUM bank limit)

last_exec_ns = None


def _split_multi_waits(nc):
    """Walrus in this container rejects instructions carrying more than one
    sync wait ("Too many sync wait commands", e.g. Drain and Ldweights
    encodings).  Re-park all but one wait of every instruction on same-engine
    NoOps inserted just before it."""
    n = 0
    for fn in nc.m.functions:
        for bb in fn.blocks:
            out = []
            for inst in bb.instructions:
                si = inst.sync_info
                waits = list(si.on_wait) if (si and si.on_wait) else []
                if len(waits) > 1:
                    si.on_wait = waits[-1:]
                    for w in waits[:-1]:
                        nop = mybir.InstNoOp(name=f"WSPLIT-{n}", ins=[], outs=[])
                        n += 1
                        nop.engine = inst.engine
                        nop.sync_info = mybir.SyncInfo(on_update=[], on_wait=[w])
                        out.append(nop)
                out.append(inst)
            bb.instructions = out


def _build_nc(mm_dtype=f32r, c=None, hp_bufs=3, hp2_bufs=6, yp_bufs=3,
              ps_bufs=2):
    nc = bass.Bass()
    xt = nc.dram_tensor("xt", [4, 18, S], mm_dtype, kind="ExternalInput")
    w1 = nc.dram_tensor("w1", [128, 98], mm_dtype, kind="ExternalInput")
    w2 = nc.dram_tensor("w2", [98, 98], mm_dtype, kind="ExternalInput")
    w3x = nc.dram_tensor("w3x", [98, 4, 72], mm_dtype, kind="ExternalInput")
    b1 = nc.dram_tensor("b1", [98, 1], f32, kind="ExternalInput")
    b2 = nc.dram_tensor("b2", [98, 1], f32, kind="ExternalInput")
    b3 = nc.dram_tensor("b3", [72, 1], f32, kind="ExternalInput")
    yt = nc.dram_tensor("yt", [72, S], f32, kind="ExternalOutput")

    relu = mybir.ActivationFunctionType.Relu
    ident = mybir.ActivationFunctionType.Identity
    add = mybir.AluOpType.add
    amax = mybir.AluOpType.max

    C = c or globals()["C"]
    with tile.TileContext(nc) as tc:
        with (
            tc.tile_pool(name="consts", bufs=1) as cp,
            tc.tile_pool(name="hid", bufs=hp_bufs) as hp,
            tc.tile_pool(name="hid2", bufs=hp2_bufs) as hp2,
            tc.tile_pool(name="out", bufs=yp_bufs) as yp,
            tc.tile_pool(name="psum", bufs=ps_bufs, space=bass.MemorySpace.PSUM) as pp,
        ):
            w1t = cp.tile([128, 98], mm_dtype)
            nc.sync.dma_start(w1t[:], w1[:])
            w2t = cp.tile([98, 98], mm_dtype)
            nc.sync.dma_start(w2t[:], w2[:])
            w3t = cp.tile([98, 4, 72], mm_dtype)
            nc.sync.dma_start(w3t[:], w3x[:])
            b1t = cp.tile([98, 1], f32)
            nc.sync.dma_start(b1t[:], b1[:])
            b2t = cp.tile([98, 1], f32)
            nc.sync.dma_start(b2t[:], b2[:])
            b3t = cp.tile([72, 1], f32)
            nc.sync.dma_start(b3t[:], b3[:])

            # Static double-buffered x tiles: memset once so the band gap
            # partitions (32j+18..32j+31) stay zero forever; the matching
            # weight rows are zero too, so any round-up reads contribute 0.
            xtiles = []
            for i in range(2):
                xtl = cp.tile([128, C], mm_dtype, tag=f"x{i}")
                nc.vector.memset(xtl[:].bitcast(f32), 0.0)
                xtiles.append(xtl)

            for s_i, c0 in enumerate(range(0, S, C)):
                xtile = xtiles[s_i % 2]
                for j in range(4):
                    nc.sync.dma_start(
                        xtile[32 * j : 32 * j + 18, :], xt[j, :, c0 : c0 + C]
                    )

                h2s = []
                for j in range(4):
                    p1 = pp.tile([98, C], f32, tag="ps1")
                    for m in range(0, C, MM):
                        kw = {"tile_position": (96, 0)} if j == 3 else {}
                        nc.tensor.matmul(
                            p1[:, m : m + MM],
                            w1t[32 * j : 32 * j + 18, :],
                            xtile[32 * j : 32 * j + 18, m : m + MM],
                            start=True,
                            stop=True,
                            **kw,
                        )
                    h1 = hp.tile([98, C], mm_dtype, tag="h1")
                    nc.scalar.activation(h1[:], p1[:], relu, bias=b1t[:, 0:1])

                    p2 = pp.tile([98, C], f32, tag="ps2")
                    for m in range(0, C, MM):
                        nc.tensor.matmul(
                            p2[:, m : m + MM],
                            w2t[:],
                            h1[:, m : m + MM],
                            start=True,
                            stop=True,
                        )
                    h2 = hp2.tile([98, C], mm_dtype, tag="h2")
                    nc.vector.tensor_scalar(
                        h2[:], p2[:], b2t[:, 0:1], 0.0, add, amax
                    )
                    h2s.append(h2)

                p3 = pp.tile([72, C], f32, tag="ps1")
                for m in range(0, C, MM):
                    for j in range(4):
                        nc.tensor.matmul(
                            p3[:, m : m + MM],
                            w3t[:, j, :],
                            h2s[j][:, m : m + MM],
                            start=(j == 0),
                            stop=(j == 3),
                        )
                ytile = yp.tile([72, C], f32, tag="y")
                nc.scalar.activation(
                    ytile[:, 0:MM], p3[:, 0:MM], ident, bias=b3t[:, 0:1]
                )
                nc.vector.tensor_scalar(
                    ytile[:, MM:C], p3[:, MM:C], b3t[:, 0:1], None, add
                )
                nc.sync.dma_start(yt[:, c0 : c0 + C], ytile[:])
    _split_multi_waits(nc)
    return nc


_nc_cache = {}


def _get_nc(mm_dtype):
    key = str(mm_dtype)
    if key not in _nc_cache:
        _nc_cache[key] = _build_nc(mm_dtype)
    return _nc_cache[key]


def _expand(pattern, params):
    pattern = np.asarray(pattern)
    params = np.asarray(params, np.float32)
    return np.where(pattern > 0, params[np.maximum(pattern - 1, 0)], 0.0).astype(
        np.float32
    )


def _blockdiag(a, b):
    out = np.zeros((a.shape[0] + b.shape[0], a.shape[1] + b.shape[1]), np.float32)
    out[: a.shape[0], : a.shape[1]] = a
    out[a.shape[0] :, a.shape[1] :] = b
    return out


def kernel(**inputs):
    global last_exec_ns
    x = np.ascontiguousarray(np.asarray(inputs["x"], np.float32))
    W1 = _expand(inputs["wp1"], inputs["w1"])  # [9, 49]
    W2 = _expand(inputs["wp2"], inputs["w2"])  # [49, 49]
    W3 = _expand(inputs["wp3"], inputs["w3"])  # [49, 9]
    B1 = _expand(inputs["bp1"], inputs["b1"])  # [49]
    B2 = _expand(inputs["bp2"], inputs["b2"])  # [49]
    B3 = _expand(inputs["bp3"], inputs["b3"])  # [9]

    w1p = _blockdiag(W1, W1)                   # [18, 98]
    w1full = np.zeros((128, 98), np.float32)
    for j in range(4):
        w1full[32 * j : 32 * j + 18] = w1p
    w2p = np.ascontiguousarray(_blockdiag(W2, W2))   # [98, 98]
    w3p = _blockdiag(W3, W3)                   # [98, 18]
    w3x = np.zeros((98, 4, 72), np.float32)
    for j in range(4):
        w3x[:, j, 18 * j : 18 * j + 18] = w3p
    b1p = np.ascontiguousarray(np.concatenate([B1, B1])[:, None])  # [98, 1]
    b2p = np.ascontiguousarray(np.concatenate([B2, B2])[:, None])  # [98, 1]
    b3p = np.ascontiguousarray(np.tile(B3, 8)[:, None])            # [72, 1]

    xT = x.T  # [9, BATCH] view
    in_maps = []
    for c in range(N_CORES):
        xc = xT[:, c * BS : (c + 1) * BS]          # [9, BS]
        xt4 = np.ascontiguousarray(
            xc.reshape(9, NBLK, S).transpose(1, 0, 2).reshape(4, 18, S)
        )
        in_maps.append(
            {
                "xt": xt4,
                "w1": w1full,
                "w2": w2p,
                "w3x": w3x,
                "b1": b1p,
                "b2": b2p,
                "b3": b3p,
            }
        )

    use_f32 = os.environ.get("KERNEL_DTYPE", "f32r") == "f32"
    nc = _get_nc(f32 if use_f32 else f32r)
    trace = os.environ.get("KERNEL_TRACE", "0") == "1"
    # The axon-tunneled NRT intermittently fails with
    # NRT_EXEC_UNIT_UNRECOVERABLE; a plain retry recovers it.
    last_err = None
    for attempt in range(4):
        try:
            res = run_bass_kernel_spmd(
                nc, in_maps, core_ids=list(range(N_CORES)), trace=trace
            )
            break
        except Exception as e:  # noqa: BLE001
            last_err = e
            import time as _time

            _time.sleep(2.0 * (attempt + 1))
    else:
        raise last_err
    if trace:
        last_exec_ns = res.exec_time_ns

    y = np.empty((BATCH, 9), np.float32)
    for c in range(N_CORES):
        ytc = res.results[c]["yt"]  # [72, S]
        # row 18j + 9h + f  <->  block k=2j+h, feature f
        yTc = ytc.reshape(4, 2, 9, S).transpose(2, 0, 1, 3).reshape(9, BS)
        y[c * BS : (c + 1) * BS] = yTc.T
    return y



# revision 2
# speedup vs baseline: 1.4959x; 1.4959x over previous
"""Equivariant MLP (9 -> 49 -> 49 -> 9, tied weights) on 8 trn2 NeuronCores.

Data parallel over the batch (1048576 rows -> 131072/core).  Tied-weight
patterns are expanded to dense matrices on the host.  Samples are processed
in PAIRS: x^T arrives as [19, 65536] bf16 (rows 0-17 = two samples' features
stacked, row 18 = ones) and every layer's bias is folded into the matmul via
the ones row, which each weight matrix propagates (extra unit column) so no
engine ever adds a bias.

Per 1024-pair iteration:
  L1  PE   [19,99]w  x [19,1024]   -> psum1 [99,1024]   (bias via ones row)
  h1  ACT  relu(psum1) -> sbuf bf16 [99,1024]           (ones row survives)
  L2  PE   [99,99]w x h1           -> psum2 [99,1024]
  h2  DVE  max(psum2,0) -> sbuf bf16 [99,1024]
  L3  PE   FLIPPED: stationary = h2 128-col chunk, moving = [99,18] weights
           -> psum3 [128 pairs, 18] per chunk; 8 chunks = [128,144].
           Ldweights is free, so L3 costs 144 PE columns instead of 1024.
  y   ACT  copy psum3 -> sbuf f32, DMA out every 8 iterations.

Emission is software-pipelined (stage offsets 0/1/1/2/3/4) so the in-order
PE queue never waits behind a dependent op; PSUM is one 4-slot rotation
(p1,p2,p3 share a tag) which exactly fills the 8 banks and whose slot-reuse
hazards are all implied by existing RAW dependencies.
"""

import os
import sys

sys.path.insert(0, "/opt/trn_rl_repo")

import numpy as np
import ml_dtypes

import concourse.bass as bass
import concourse.mybir as mybir
import concourse.tile as tile
from concourse.bass_utils import run_bass_kernel_spmd

f32 = mybir.dt.float32
bf16 = mybir.dt.bfloat16

N_CORES = 8
BATCH = 1048576
BS = BATCH // N_CORES          # 131072 samples per core
NPAIR = BS // 2                # 65536 pair columns per core
C = 1024                       # pair columns per iteration
NITER = NPAIR // C             # 64
XCH = 8                        # iterations per x DMA chunk
YCH = 8                        # iterations per y DMA chunk
MM = 512                       # matmul moving width (one PSUM bank)

last_exec_ns = None


def _split_multi_waits(nc):
    """Walrus in this container rejects instructions carrying more than one
    sync wait ("Too many sync wait commands").  Re-park all but one wait of
    every instruction on same-engine NoOps inserted just before it."""
    n = 0
    for fn in nc.m.functions:
        for bb in fn.blocks:
            out = []
            for inst in bb.instructions:
                si = inst.sync_info
                waits = list(si.on_wait) if (si and si.on_wait) else []
                if len(waits) > 1:
                    si.on_wait = waits[-1:]
                    for w in waits[:-1]:
                        nop = mybir.InstNoOp(name=f"WSPLIT-{n}", ins=[], outs=[])
                        n += 1
                        nop.engine = inst.engine
                        nop.sync_info = mybir.SyncInfo(on_update=[], on_wait=[w])
                        out.append(nop)
                out.append(inst)
            bb.instructions = out
    return nc


def _build_nc():
    nc = bass.Bass()
    xt = nc.dram_tensor("xt", [19, NPAIR], bf16, kind="ExternalInput")
    w1 = nc.dram_tensor("w1", [19, 99], bf16, kind="ExternalInput")
    w2 = nc.dram_tensor("w2", [99, 99], bf16, kind="ExternalInput")
    w3 = nc.dram_tensor("w3", [99, 18], bf16, kind="ExternalInput")
    yt = nc.dram_tensor("yt", [NITER // YCH, 128, YCH * 144], f32,
                        kind="ExternalOutput")

    relu = mybir.ActivationFunctionType.Relu
    amax = mybir.AluOpType.max
    XW = XCH * C                   # pair columns per x chunk

    with tile.TileContext(nc) as tc:
        with (
            tc.tile_pool(name="consts", bufs=1) as cp,
            tc.tile_pool(name="xp", bufs=2) as xp,
            tc.tile_pool(name="hid", bufs=2) as hp,
            tc.tile_pool(name="yp", bufs=2) as yp,
            tc.tile_pool(name="ps", bufs=4, space=bass.MemorySpace.PSUM) as pp,
        ):
            w1t = cp.tile([19, 99], bf16)
            nc.sync.dma_start(w1t[:], w1[:])
            w2t = cp.tile([99, 99], bf16)
            nc.sync.dma_start(w2t[:], w2[:])
            w3t = cp.tile([99, 18], bf16)
            nc.sync.dma_start(w3t[:], w3[:])

            xts = {}

            def xdma(g):
                xts[g] = xp.tile([19, XW], bf16, tag="xts", name=f"xts{g}")
                nc.sync.dma_start(xts[g][:], xt[:, g * XW:(g + 1) * XW])

            xdma(0)

            p1s, p2s, p3s, h1s, h2s = {}, {}, {}, {}, {}
            ycur = [None]

            for i in range(NITER + 4):
                if i % XCH == 0 and i // XCH + 1 < NITER // XCH:
                    xdma(i // XCH + 1)

                if i < NITER:                       # L1(i) on PE
                    p1 = pp.tile([99, C], f32, tag="ps", name=f"p1_{i}")
                    src = xts[i // XCH]
                    off = (i % XCH) * C
                    for m in range(0, C, MM):
                        nc.tensor.matmul(
                            p1[:, m:m + MM], w1t[:],
                            src[:, off + m:off + m + MM],
                            start=True, stop=True)
                    p1s[i] = p1

                if 1 <= i <= NITER:                 # h1(i-1) on ACT
                    k = i - 1
                    h1 = hp.tile([99, C], bf16, tag="h1", name=f"h1_{k}")
                    nc.scalar.activation(h1[:], p1s.pop(k)[:], relu)
                    h1s[k] = h1

                if 1 <= i <= NITER:                 # L2(i-1) on PE
                    k = i - 1
                    p2 = pp.tile([99, C], f32, tag="ps", name=f"p2_{k}")
                    h1 = h1s.pop(k)
                    for m in range(0, C, MM):
                        nc.tensor.matmul(
                            p2[:, m:m + MM], w2t[:], h1[:, m:m + MM],
                            start=True, stop=True)
                    p2s[k] = p2

                if 2 <= i <= NITER + 1:             # h2(i-2) on DVE
                    k = i - 2
                    h2 = hp.tile([99, C], bf16, tag="h2", name=f"h2_{k}")
                    nc.vector.tensor_scalar(h2[:], p2s.pop(k)[:], 0.0, None,
                                            amax)
                    h2s[k] = h2

                if 3 <= i <= NITER + 2:             # L3(i-3) on PE, flipped
                    k = i - 3
                    p3 = pp.tile([128, 144], f32, tag="ps", name=f"p3_{k}")
                    h2 = h2s.pop(k)
                    for c in range(8):
                        nc.tensor.matmul(
                            p3[:, c * 18:(c + 1) * 18],
                            h2[:, c * 128:(c + 1) * 128], w3t[:],
                            start=True, stop=True)
                    p3s[k] = p3

                if 4 <= i <= NITER + 3:             # y(i-4) on ACT + DMA
                    k = i - 4
                    q = k % YCH
                    if q == 0:
                        ycur[0] = yp.tile([128, YCH * 144], f32, tag="yt",
                                          name=f"y_{k // YCH}")
                    nc.scalar.copy(ycur[0][:, q * 144:(q + 1) * 144],
                                   p3s.pop(k)[:])
                    if q == YCH - 1:
                        nc.sync.dma_start(yt[k // YCH], ycur[0][:])

    return _split_multi_waits(nc)


_nc_cache = {}


def _get_nc(*_ignored):
    if "nc" not in _nc_cache:
        _nc_cache["nc"] = _build_nc()
    return _nc_cache["nc"]


def _expand(pattern, params):
    pattern = np.asarray(pattern)
    params = np.asarray(params, np.float32)
    return np.where(pattern > 0, params[np.maximum(pattern - 1, 0)], 0.0).astype(
        np.float32
    )


def _blockdiag(a):
    o = np.zeros((2 * a.shape[0], 2 * a.shape[1]), np.float32)
    o[:a.shape[0], :a.shape[1]] = a
    o[a.shape[0]:, a.shape[1]:] = a
    return o


def kernel(**inputs):
    global last_exec_ns
    x = np.ascontiguousarray(np.asarray(inputs["x"], np.float32))
    W1 = _expand(inputs["wp1"], inputs["w1"])  # [9, 49]
    W2 = _expand(inputs["wp2"], inputs["w2"])  # [49, 49]
    W3 = _expand(inputs["wp3"], inputs["w3"])  # [49, 9]
    B1 = _expand(inputs["bp1"], inputs["b1"])  # [49]
    B2 = _expand(inputs["bp2"], inputs["b2"])  # [49]
    B3 = _expand(inputs["bp3"], inputs["b3"])  # [9]

    w1e = np.zeros((19, 99), np.float32)
    w1e[0:18, 0:98] = _blockdiag(W1)
    w1e[18, 0:98] = np.concatenate([B1, B1])
    w1e[18, 98] = 1.0
    w2e = np.zeros((99, 99), np.float32)
    w2e[0:98, 0:98] = _blockdiag(W2)
    w2e[98, 0:98] = np.concatenate([B2, B2])
    w2e[98, 98] = 1.0
    w3e = np.zeros((99, 18), np.float32)
    w3e[0:98, :] = _blockdiag(W3)
    w3e[98, :] = np.concatenate([B3, B3])

    w1b = w1e.astype(ml_dtypes.bfloat16)
    w2b = w2e.astype(ml_dtypes.bfloat16)
    w3b = w3e.astype(ml_dtypes.bfloat16)

    ones = np.ones((1, NPAIR), np.float32)
    in_maps = []
    for c in range(N_CORES):
        xc = x[c * BS:(c + 1) * BS]                       # [BS, 9]
        xpair = xc.reshape(NPAIR, 18).T                   # [18, NPAIR] view
        xfull = np.concatenate([xpair, ones], axis=0)     # [19, NPAIR]
        in_maps.append({
            "xt": np.ascontiguousarray(xfull).astype(ml_dtypes.bfloat16),
            "w1": w1b, "w2": w2b, "w3": w3b,
        })

    nc = _get_nc()
    trace = os.environ.get("KERNEL_TRACE", "0") == "1"
    # The axon-tunneled NRT intermittently fails with
    # NRT_EXEC_UNIT_UNRECOVERABLE; a plain retry recovers it.
    last_err = None
    for attempt in range(4):
        try:
            res = run_bass_kernel_spmd(
                nc, in_maps, core_ids=list(range(N_CORES)), trace=trace
            )
            break
        except Exception as e:  # noqa: BLE001
            last_err = e
            import time as _time

            _time.sleep(2.0 * (attempt + 1))
    else:
        raise last_err
    if trace:
        last_exec_ns = res.exec_time_ns

    y = np.empty((BATCH, 9), np.float32)
    for c in range(N_CORES):
        ytc = res.results[c]["yt"]  # [8, 128, 1152]
        # ytc[g, n, q*144 + cc*18 + h*9 + f] -> sample 2*(((g*8+q)*8+cc)*128+n)+h
        arr = ytc.reshape(8, 128, YCH, 8, 2, 9).transpose(0, 2, 3, 1, 4, 5)
        y[c * BS:(c + 1) * BS] = arr.reshape(BS, 9)
    return y


# revision 4
# speedup vs baseline: 1.5310x; 1.0235x over previous
"""Equivariant MLP (9 -> 49 -> 49 -> 9, tied weights) on 8 trn2 NeuronCores.

Data parallel over the batch (1048576 rows -> 131072/core).  Tied-weight
patterns are expanded to dense matrices on the host.  Samples are processed
in PAIRS: x^T arrives as [19, 65536] bf16 (rows 0-17 = two samples' features
stacked, row 18 = ones) and every layer's bias is folded into the matmul via
the ones row, which each weight matrix propagates (extra unit column) so no
engine ever adds a bias.

Per 1024-pair iteration:
  L1  PE   [19,99]w  x [19,1024]   -> psum1 [99,1024]   (bias via ones row)
  h1  ACT  relu(psum1) -> sbuf bf16 [99,1024]           (ones row survives)
  L2  PE   [99,99]w x h1           -> psum2 [99,1024]
  h2  DVE  max(psum2,0) -> sbuf bf16 [99,1024]
  L3  PE   FLIPPED: stationary = h2 128-col chunk, moving = [99,18] weights
           -> psum3 [128 pairs, 18] per chunk; 8 chunks = [128,144].
           Ldweights is free, so L3 costs 144 PE columns instead of 1024.
  y   ACT  copy psum3 -> sbuf f32, DMA out every 8 iterations.

Emission is software-pipelined (stage offsets 0/1/1/2/3/4) so the in-order
PE queue never waits behind a dependent op; PSUM is one 4-slot rotation
(p1,p2,p3 share a tag) which exactly fills the 8 banks and whose slot-reuse
hazards are all implied by existing RAW dependencies.
"""

import os
import sys

sys.path.insert(0, "/opt/trn_rl_repo")

import numpy as np
import ml_dtypes

import concourse.bass as bass
import concourse.mybir as mybir
import concourse.tile as tile
from concourse.bass_utils import run_bass_kernel_spmd

f32 = mybir.dt.float32
bf16 = mybir.dt.bfloat16

N_CORES = 8
BATCH = 1048576
BS = BATCH // N_CORES          # 131072 samples per core
NPAIR = BS // 2                # 65536 pair columns per core
C = 1024                       # pair columns per iteration
NITER = NPAIR // C             # 64
XCH = 8                        # iterations per x DMA chunk
YCH = 8                        # iterations per y DMA chunk
MM = 512                       # matmul moving width (one PSUM bank)

last_exec_ns = None


def _split_multi_waits(nc):
    """Walrus in this container rejects instructions carrying more than one
    sync wait ("Too many sync wait commands").  Re-park all but one wait of
    every instruction on same-engine NoOps inserted just before it."""
    n = 0
    for fn in nc.m.functions:
        for bb in fn.blocks:
            out = []
            for inst in bb.instructions:
                si = inst.sync_info
                waits = list(si.on_wait) if (si and si.on_wait) else []
                if len(waits) > 1:
                    si.on_wait = waits[-1:]
                    for w in waits[:-1]:
                        nop = mybir.InstNoOp(name=f"WSPLIT-{n}", ins=[], outs=[])
                        n += 1
                        nop.engine = inst.engine
                        nop.sync_info = mybir.SyncInfo(on_update=[], on_wait=[w])
                        out.append(nop)
                out.append(inst)
            bb.instructions = out
    return nc


def _build_nc():
    nc = bass.Bass()
    xt = nc.dram_tensor("xt", [19, NPAIR], bf16, kind="ExternalInput")
    w1 = nc.dram_tensor("w1", [19, 99], bf16, kind="ExternalInput")
    w2 = nc.dram_tensor("w2", [99, 99], bf16, kind="ExternalInput")
    w3 = nc.dram_tensor("w3", [99, 18], bf16, kind="ExternalInput")
    yt = nc.dram_tensor("yt", [NITER // YCH, 128, YCH * 144], f32,
                        kind="ExternalOutput")

    relu = mybir.ActivationFunctionType.Relu
    amax = mybir.AluOpType.max
    XW = XCH * C                   # pair columns per x chunk

    with tile.TileContext(nc) as tc:
        with (
            tc.tile_pool(name="consts", bufs=1) as cp,
            tc.tile_pool(name="xp", bufs=2) as xp,
            tc.tile_pool(name="hid", bufs=2) as hp,
            tc.tile_pool(name="yp", bufs=2) as yp,
            tc.tile_pool(name="ps", bufs=3, space=bass.MemorySpace.PSUM) as pp,
        ):
            w1t = cp.tile([19, 99], bf16)
            nc.sync.dma_start(w1t[:], w1[:])
            w2t = cp.tile([99, 99], bf16)
            nc.sync.dma_start(w2t[:], w2[:])
            w3t = cp.tile([99, 18], bf16)
            nc.sync.dma_start(w3t[:], w3[:])

            xts = {}

            def xdma(g):
                xts[g] = xp.tile([19, XW], bf16, tag="xts", name=f"xts{g}")
                nc.sync.dma_start(xts[g][:], xt[:, g * XW:(g + 1) * XW])

            xdma(0)

            p1s, p2s, p3s, h1s, h2s = {}, {}, {}, {}, {}
            ycur = [None]

            for i in range(NITER + 5):
                if i % XCH == 0 and i // XCH + 1 < NITER // XCH:
                    xdma(i // XCH + 1)

                if i < NITER:                       # L1(i) on PE
                    p1 = pp.tile([99, C], f32, tag="ps", name=f"p1_{i}")
                    src = xts[i // XCH]
                    off = (i % XCH) * C
                    for m in range(0, C, MM):
                        nc.tensor.matmul(
                            p1[:, m:m + MM], w1t[:],
                            src[:, off + m:off + m + MM],
                            start=True, stop=True)
                    p1s[i] = p1

                if 1 <= i <= NITER:                 # h1(i-1) on ACT
                    k = i - 1
                    h1 = hp.tile([99, C], bf16, tag="h1", name=f"h1_{k}")
                    nc.scalar.activation(h1[:], p1s.pop(k)[:], relu)
                    h1s[k] = h1

                if 1 <= i <= NITER:                 # L2(i-1) on PE
                    k = i - 1
                    p2 = pp.tile([99, C], f32, tag="ps", name=f"p2_{k}")
                    h1 = h1s.pop(k)
                    for m in range(0, C, MM):
                        nc.tensor.matmul(
                            p2[:, m:m + MM], w2t[:], h1[:, m:m + MM],
                            start=True, stop=True)
                    p2s[k] = p2

                if 1 <= i <= NITER:                 # h2(i-1) on DVE, same round
                    k = i - 1
                    h2 = hp.tile([99, C], bf16, tag="h2", name=f"h2_{k}")
                    nc.vector.tensor_scalar(h2[:], p2s.pop(k)[:], 0.0, None,
                                            amax)
                    h2s[k] = h2

                if 2 <= i <= NITER + 1:             # L3(i-2) on PE, flipped
                    k = i - 2
                    if k % 2 == 0:
                        p3s[k // 2] = pp.tile([128, 288], f32, tag="p3",
                                              bufs=2, name=f"p3_{k // 2}")
                    p3 = p3s[k // 2]
                    h2 = h2s.pop(k)
                    base = (k % 2) * 144
                    for c in range(8):
                        nc.tensor.matmul(
                            p3[:, base + c * 18:base + (c + 1) * 18],
                            h2[:, c * 128:(c + 1) * 128], w3t[:],
                            start=True, stop=True)

                if 4 <= i <= NITER + 3 and (i - 4) % 2 == 0:
                    k = i - 4                       # y group (k, k+1) on ACT
                    g = k // 2
                    q = k % YCH
                    if q == 0:
                        ycur[0] = yp.tile([128, YCH * 144], f32, tag="yt",
                                          name=f"y_{k // YCH}")
                    nc.scalar.copy(ycur[0][:, q * 144:(q + 2) * 144],
                                   p3s.pop(g)[:])
                    if q == YCH - 2:
                        nc.sync.dma_start(yt[k // YCH], ycur[0][:])

    return _split_multi_waits(nc)


_nc_cache = {}


def _get_nc(*_ignored):
    if "nc" not in _nc_cache:
        _nc_cache["nc"] = _build_nc()
    return _nc_cache["nc"]


def _expand(pattern, params):
    pattern = np.asarray(pattern)
    params = np.asarray(params, np.float32)
    return np.where(pattern > 0, params[np.maximum(pattern - 1, 0)], 0.0).astype(
        np.float32
    )


def _blockdiag(a):
    o = np.zeros((2 * a.shape[0], 2 * a.shape[1]), np.float32)
    o[:a.shape[0], :a.shape[1]] = a
    o[a.shape[0]:, a.shape[1]:] = a
    return o


def kernel(**inputs):
    global last_exec_ns
    x = np.ascontiguousarray(np.asarray(inputs["x"], np.float32))
    W1 = _expand(inputs["wp1"], inputs["w1"])  # [9, 49]
    W2 = _expand(inputs["wp2"], inputs["w2"])  # [49, 49]
    W3 = _expand(inputs["wp3"], inputs["w3"])  # [49, 9]
    B1 = _expand(inputs["bp1"], inputs["b1"])  # [49]
    B2 = _expand(inputs["bp2"], inputs["b2"])  # [49]
    B3 = _expand(inputs["bp3"], inputs["b3"])  # [9]

    w1e = np.zeros((19, 99), np.float32)
    w1e[0:18, 0:98] = _blockdiag(W1)
    w1e[18, 0:98] = np.concatenate([B1, B1])
    w1e[18, 98] = 1.0
    w2e = np.zeros((99, 99), np.float32)
    w2e[0:98, 0:98] = _blockdiag(W2)
    w2e[98, 0:98] = np.concatenate([B2, B2])
    w2e[98, 98] = 1.0
    w3e = np.zeros((99, 18), np.float32)
    w3e[0:98, :] = _blockdiag(W3)
    w3e[98, :] = np.concatenate([B3, B3])

    w1b = w1e.astype(ml_dtypes.bfloat16)
    w2b = w2e.astype(ml_dtypes.bfloat16)
    w3b = w3e.astype(ml_dtypes.bfloat16)

    ones = np.ones((1, NPAIR), np.float32)
    in_maps = []
    for c in range(N_CORES):
        xc = x[c * BS:(c + 1) * BS]                       # [BS, 9]
        xpair = xc.reshape(NPAIR, 18).T                   # [18, NPAIR] view
        xfull = np.concatenate([xpair, ones], axis=0)     # [19, NPAIR]
        in_maps.append({
            "xt": np.ascontiguousarray(xfull).astype(ml_dtypes.bfloat16),
            "w1": w1b, "w2": w2b, "w3": w3b,
        })

    nc = _get_nc()
    trace = os.environ.get("KERNEL_TRACE", "0") == "1"
    # The axon-tunneled NRT intermittently fails with
    # NRT_EXEC_UNIT_UNRECOVERABLE; a plain retry recovers it.
    last_err = None
    for attempt in range(4):
        try:
            res = run_bass_kernel_spmd(
                nc, in_maps, core_ids=list(range(N_CORES)), trace=trace
            )
            break
        except Exception as e:  # noqa: BLE001
            last_err = e
            import time as _time

            _time.sleep(2.0 * (attempt + 1))
    else:
        raise last_err
    if trace:
        last_exec_ns = res.exec_time_ns

    y = np.empty((BATCH, 9), np.float32)
    for c in range(N_CORES):
        ytc = res.results[c]["yt"]  # [8, 128, 1152]
        # ytc[g, n, q*144 + cc*18 + h*9 + f] -> sample 2*(((g*8+q)*8+cc)*128+n)+h
        arr = ytc.reshape(8, 128, YCH, 8, 2, 9).transpose(0, 2, 3, 1, 4, 5)
        y[c * BS:(c + 1) * BS] = arr.reshape(BS, 9)
    return y
